# revision 36
# baseline (speedup 1.0000x reference)
"""Causal multi-head attention (B=1, S=2048, H=16, D=128, fp32) on 8 TRN2
NeuronCores — 67-69us HW exec, rel err ~3.8e-4 vs fp32 reference.

Sharding: pure head parallelism — 16 heads / 8 cores = 2 heads per core, no
collectives (beats ring+Ulysses at this size: zero comm, perfectly balanced
causal work).  Each core receives its 2 heads' Q/K pre-transposed on host to
[h, d, s] fp16 (contraction dim on partitions, clean DMA lines), V natural
[s, h, d] fp16, and returns its output transposed [h, d, s] fp32 (host
transposes back).  fp16 runs the PE at the same 1 cycle/row as bf16 but
carries a 10-bit mantissa, so accuracy lands near f32r at twice its speed.

Per-core kernel (per head, s-blocks of 512, the two heads' group streams
interleaved so ACT exp latency hides under the other head's PE work, with a
one-group software-pipeline lookahead):
  - scores^T pair = [K^T tile_i | tile_i+1].T @ Q^T block -> one 2-bank PSUM
    tile [t=128, 2, s<=512]
  - one batched exp on ACT per pair (scale 1/sqrt(D) fused), fp16 out
  - causal diagonal via static upper-triangular 0/1 mask mult on GpSimd
  - O^T  += V_tile.T @ expT        (fp16 matmuls, accumulated per t tile)
  - denominator l: full tiles partial-summed on DVE (fp16 pair adds ->
    f32r accumulate), diagonal tiles summed on PE via ones-matmuls, then one
    ones-matmul per block contracts the DVE partials over the partition dim
  - normalize O^T * reciprocal_approx_fast(l) on DVE, DMA out [d, s].
Causality skips fully-masked tiles and shrinks diagonal-crossing tiles; K/Q/V
are chunked per 512 columns and DMA'd in consumption order (first transfers
split across two queues) so compute starts ~10us in instead of after the
full load; blocks run (1,2,3,0) so the tail ends on the smallest block.
"""

import math

import numpy as np

import concourse.mybir as mybir
import concourse.tile as tile
from concourse import bacc
from concourse.masks import make_upper_triangular

S = 2048
H = 16
D = 128
HC = 2  # heads per core
NCORES = 8
P = 128
SBLK = 512  # s-block width
NT = S // P  # 16 t tiles
NB = S // SBLK  # 4 s blocks / chunks
TPB = SBLK // P  # 4 t tiles per s block
SCALE = 1.0 / math.sqrt(D)

F32 = mybir.dt.float32
F32R = mybir.dt.float32r
BF16 = mybir.dt.float16  # fp16: same PE rate as bf16, 10-bit mantissa

# mm1 (QK^T) precision: False -> f32r (fp32 inputs), True -> bf16
MM1_BF16 = True


def build_nc(mm1_bf16=MM1_BF16):
    qk_np = np.float32 if not mm1_bf16 else np.float16
    qk_dt = F32 if not mm1_bf16 else BF16
    qk_sb_dt = F32R if not mm1_bf16 else BF16

    nc = bacc.Bacc("TRN2", target_bir_lowering=False, debug=False, num_devices=NCORES)
    qt_d = nc.dram_tensor("qt", [HC, D, S], qk_dt, kind="ExternalInput").ap()
    kt_d = nc.dram_tensor("kt", [HC, D, S], qk_dt, kind="ExternalInput").ap()
    v_d = nc.dram_tensor("v", [S, HC, D], BF16, kind="ExternalInput").ap()
    ot_d = nc.dram_tensor("ot", [HC, D, S], F32, kind="ExternalOutput").ap()

    with tile.TileContext(nc) as tc:
        with (
            tc.tile_pool(name="consts", bufs=1) as cpool,
            tc.tile_pool(name="big", bufs=1) as bigpool,
            tc.tile_pool(name="exp", bufs=8) as epool,
            tc.tile_pool(name="norm", bufs=3) as npool,
            tc.tile_pool(name="esum", bufs=3) as espool,
            tc.tile_pool(name="psum_s", bufs=2, space="PSUM") as ps_pool,
            tc.tile_pool(name="psum_o", bufs=3, space="PSUM") as po_pool,
            tc.tile_pool(name="psum_l", bufs=1, space="PSUM") as pl_pool,
        ):
            ones = cpool.tile([P, P], BF16, tag="ones")
            nc.vector.memset(ones, 1.0)
            warm_ps = pl_pool.tile([P, SBLK], F32, tag="pl", name="warm_ps")
            for w in range(40):
                nc.tensor.matmul(
                    warm_ps[:, :P],
                    ones[:],
                    ones[:],
                    start=True,
                    stop=True,
                    skip_group_check=True,
                )
            ones_f = cpool.tile([P, P], F32, tag="ones_f")
            nc.vector.memset(ones_f, 1.0)
            ones_r = cpool.tile([P, P], F32R, tag="ones_r")
            nc.vector.tensor_copy(out=ones_r[:], in_=ones_f[:])
            tri = cpool.tile([P, P], BF16, tag="tri")
            make_upper_triangular(nc, tri, val=1.0, diag=True)

            # chunked SBUF inputs: per-head K^T/Q^T [d, 512] chunks (qk_sb_dt)
            # and V natural [t-part, j, h, d] bf16 chunks, loaded in the order
            # compute consumes them.
            kt_c = {}
            qt_c = {}
            vb_c = {}
            vre = v_d.rearrange("(i p) h d -> p i h d", p=P)
            for c in range(NB):
                for h in range(HC):
                    kt_c[h, c] = bigpool.tile(
                        [P, SBLK], qk_sb_dt, tag=f"ktc{h}_{c}", name=f"ktc{h}_{c}"
                    )
                    qt_c[h, c] = bigpool.tile(
                        [P, SBLK], qk_sb_dt, tag=f"qtc{h}_{c}", name=f"qtc{h}_{c}"
                    )
                vb_c[c] = bigpool.tile(
                    [P, TPB, HC, D], BF16, tag=f"vbc{c}", name=f"vbc{c}"
                )
            # issue DMAs in the order blocks consume them, alternating issue
            # engines so descriptor writes don't serialize on one sequencer
            dma_jobs = []
            seen = set()

            def _need(key, dst, srcap):
                if key not in seen:
                    seen.add(key)
                    dma_jobs.append((dst, srcap))

            for b in (0, 1, 2, 3):
                for h in range(HC):
                    cs = slice(b * SBLK, (b + 1) * SBLK)
                    _need(("q", h, b), qt_c[h, b], qt_d[h, :, cs].bitcast(qk_sb_dt))
                    for c in range(b + 1):
                        ks = slice(c * SBLK, (c + 1) * SBLK)
                        _need(("k", h, c), kt_c[h, c], kt_d[h, :, ks].bitcast(qk_sb_dt))
                        _need(("v", c), vb_c[c], vre[:, c * TPB : (c + 1) * TPB])
            for n_dma, (dst, srcap) in enumerate(dma_jobs):
                eng = nc.sync
                if n_dma < 5 and len(dst.shape) == 2:
                    # split the first, compute-gating transfers across two
                    # queues each so the pipeline fills sooner
                    half = dst.shape[-1] // 2
                    eng.dma_start(dst[:, :half], srcap[:, :half])
                    eng.dma_start(dst[:, half:], srcap[:, half:])
                else:
                    eng.dma_start(dst[:], srcap)

            def kt_tile(h, i):
                return kt_c[h, i // TPB][:, (i % TPB) * P : (i % TPB + 1) * P]

            def v_tile(h, i):
                return vb_c[i // TPB][:, i % TPB, h, :]

            # smallest block first: it only needs the first input chunks, so
            # the pipeline fills at the earliest possible moment
            for b in (0, 1, 2, 3):
                n_full = TPB * b  # fully-unmasked t tiles (even count)
                # groups of two t tiles sharing one 2-bank psum + one exp:
                # (i0, i1, s_lo0, s_lo1, is_diag)
                groups = [(ip, ip + 1, 0, 0, False) for ip in range(0, n_full, 2)]
                groups += [
                    (n_full, n_full + 1, 0, P, True),
                    (n_full + 2, n_full + 3, 2 * P, 3 * P, True),
                ]
                n_groups = len(groups)
                last_i = n_full + TPB - 1

                psum_o = {}
                psum_l = {}
                expsum = {}
                expt_of = {}
                for h in range(HC):
                    psum_o[h] = po_pool.tile(
                        [P, SBLK], F32, tag="po", name=f"po{h}_{b}"
                    )
                    psum_l[h] = pl_pool.tile(
                        [P, SBLK], F32, tag="pl", name=f"pl{h}_{b}"
                    )
                    if n_full:
                        expsum[h] = bigpool.tile(
                            [P, SBLK], F32R, tag=f"esum{h}_{b}", name=f"es{h}_{b}"
                        )

                def emit_mm1(h, g):
                    i0, i1, s0, s1, is_diag = groups[g]
                    psum_s = ps_pool.tile(
                        [P, 2, SBLK], F32, tag="ps", name=f"ps{h}_{b}_{g}"
                    )
                    expt = epool.tile(
                        [P, 2, SBLK], BF16, tag="expt", name=f"ex{h}_{b}_{g}"
                    )
                    for j, (i, s_lo) in enumerate(((i0, s0), (i1, s1))):
                        nc.tensor.matmul(
                            psum_s[:, j, s_lo:],
                            kt_tile(h, i),
                            qt_c[h, b][:, s_lo:],
                            start=True,
                            stop=True,
                        )
                    # one exp for both tiles; [s0:s1] of tile 1 is stale-finite
                    # psum, never read downstream
                    nc.scalar.activation(
                        expt[:, :, s0:],
                        psum_s[:, :, s0:],
                        mybir.ActivationFunctionType.Exp,
                        scale=SCALE,
                    )
                    if is_diag:
                        for j, s_lo in enumerate((s0, s1)):
                            nc.gpsimd.tensor_mul(
                                out=expt[:, j, s_lo : s_lo + P],
                                in0=expt[:, j, s_lo : s_lo + P],
                                in1=tri[:],
                            )
                    expt_of[h, g] = expt

                def emit_mm2(h, g):
                    i0, i1, s0, s1, is_diag = groups[g]
                    expt = expt_of.pop((h, g))
                    for j, (i, s_lo) in enumerate(((i0, s0), (i1, s1))):
                        nc.tensor.matmul(
                            psum_o[h][:, s_lo:],
                            v_tile(h, i),
                            expt[:, j, s_lo:],
                            start=(i == 0),
                            stop=(i == last_i),
                            skip_group_check=True,
                        )
                        if is_diag:
                            # diagonal denominator contributions on PE
                            nc.tensor.matmul(
                                psum_l[h][:, s_lo:],
                                ones[:],
                                expt[:, j, s_lo:],
                                start=(i == n_full),
                                stop=(i == last_i and n_full == 0),
                                skip_group_check=True,
                            )
                    if not is_diag:
                        # full-tile denominator contributions accumulate on DVE
                        # (same-dtype inputs per op: bf16+bf16 -> fp32 pair sum,
                        # then fp32+fp32 accumulate)
                        if i0 == 0:
                            nc.vector.tensor_add(
                                out=expsum[h][:],
                                in0=expt[:, 0, :],
                                in1=expt[:, 1, :],
                            )
                        else:
                            psum_pair = npool.tile(
                                [P, SBLK], BF16, tag="epair", name=f"ep{h}_{b}_{i0}"
                            )
                            nc.vector.tensor_add(
                                out=psum_pair[:],
                                in0=expt[:, 0, :],
                                in1=expt[:, 1, :],
                            )
                            nc.vector.tensor_add(
                                out=expsum[h][:],
                                in0=expsum[h][:],
                                in1=psum_pair[:],
                            )

                # interleave the two heads' streams: PE runs head A's mm2
                # while ACT computes head B's exp
                pending = None
                for g in range(n_groups):
                    for h in range(HC):
                        emit_mm1(h, g)
                    if pending is not None:
                        for h in range(HC):
                            emit_mm2(h, pending)
                    pending = g
                for h in range(HC):
                    emit_mm2(h, pending)

                for h in range(HC):
                    bs = slice(b * SBLK, (b + 1) * SBLK)
                    if n_full:
                        # contract the DVE partial sums over the partition dim
                        nc.tensor.matmul(
                            psum_l[h][:],
                            ones_r[:],
                            expsum[h][:],
                            start=False,
                            stop=True,
                            skip_group_check=True,
                        )
                    recip = npool.tile([P, SBLK], F32, tag="recip", name=f"rc{h}_{b}")
                    nc.vector.reciprocal_approx_fast(out=recip[:], in_=psum_l[h][:])
                    otn = npool.tile([P, SBLK], F32, tag="otn", name=f"ot{h}_{b}")
                    nc.vector.tensor_mul(out=otn[:], in0=psum_o[h][:], in1=recip[:])
                    # split across queues so the final transfer (which
                    # gates the exit drain) completes sooner; the last block
                    # gets a 4-way split
                    nsp = 4 if b == 3 else 2
                    hw = SBLK // nsp
                    lo = b * SBLK
                    for sp in range(nsp):
                        nc.sync.dma_start(
                            ot_d[h, :, lo + sp * hw : lo + (sp + 1) * hw],
                            otn[:, sp * hw : (sp + 1) * hw],
                        )
    nc.compile()
    return nc


_NC_CACHE = None


def _get_nc():
    global _NC_CACHE
    if _NC_CACHE is None:
        _NC_CACHE = build_nc()
    return _NC_CACHE


def make_in_maps(query, key, value):
    qk_np = np.float32 if not MM1_BF16 else np.float16
    query = np.asarray(query)
    key = np.asarray(key)
    value = np.asarray(value)
    in_maps = []
    for c in range(NCORES):
        hs = slice(c * HC, (c + 1) * HC)
        in_maps.append(
            {
                "qt": np.ascontiguousarray(
                    query[0, :, hs, :].transpose(1, 2, 0)
                ).astype(qk_np),
                "kt": np.ascontiguousarray(
                    key[0, :, hs, :].transpose(1, 2, 0)
                ).astype(qk_np),
                "v": np.ascontiguousarray(value[0, :, hs, :]).astype(
                    np.float16
                ),
            }
        )
    return in_maps


def kernel(query, key, value):
    from concourse.bass_utils import run_bass_kernel_spmd

    nc = _get_nc()
    in_maps = make_in_maps(query, key, value)
    res = run_bass_kernel_spmd(nc, in_maps, core_ids=list(range(NCORES)))
    out = np.empty((1, S, H, D), dtype=np.float32)
    for c in range(NCORES):
        # ot is [HC, D, S] -> [S, HC, D]
        out[0, :, c * HC : (c + 1) * HC, :] = res.results[c]["ot"].transpose(2, 0, 1)
    return out


# revision 37
# speedup vs baseline: 1.0526x; 1.0526x over previous
"""Causal multi-head attention (B=1, S=2048, H=16, D=128, fp32) on 8 TRN2
NeuronCores — 67-69us HW exec, rel err ~3.8e-4 vs fp32 reference.

Sharding: pure head parallelism — 16 heads / 8 cores = 2 heads per core, no
collectives (beats ring+Ulysses at this size: zero comm, perfectly balanced
causal work).  Each core receives its 2 heads' Q/K pre-transposed on host to
[h, d, s] fp16 (contraction dim on partitions, clean DMA lines), V natural
[s, h, d] fp16, and returns its output transposed [h, d, s] fp32 (host
transposes back).  fp16 runs the PE at the same 1 cycle/row as bf16 but
carries a 10-bit mantissa, so accuracy lands near f32r at twice its speed.

Per-core kernel (per head, s-blocks of 512, the two heads' group streams
interleaved so ACT exp latency hides under the other head's PE work, with a
one-group software-pipeline lookahead):
  - scores^T pair = [K^T tile_i | tile_i+1].T @ Q^T block -> one 2-bank PSUM
    tile [t=128, 2, s<=512]
  - one batched exp on ACT per pair (scale 1/sqrt(D) fused), fp16 out
  - causal diagonal via static upper-triangular 0/1 mask mult on GpSimd
  - O^T  += V_tile.T @ expT        (fp16 matmuls, accumulated per t tile)
  - denominator l: full tiles partial-summed on DVE (fp16 pair adds ->
    f32r accumulate), diagonal tiles summed on PE via ones-matmuls, then one
    ones-matmul per block contracts the DVE partials over the partition dim
  - normalize O^T * reciprocal_approx_fast(l) on DVE, DMA out [d, s].
Causality skips fully-masked tiles and shrinks diagonal-crossing tiles; K/Q/V
are chunked per 512 columns and DMA'd in consumption order (first transfers
split across two queues) so compute starts ~10us in instead of after the
full load; blocks run (1,2,3,0) so the tail ends on the smallest block.
"""

import math

import numpy as np

import concourse.mybir as mybir
import concourse.tile as tile
from concourse import bacc
from concourse.masks import make_upper_triangular

S = 2048
H = 16
D = 128
HC = 2  # heads per core
NCORES = 8
P = 128
SBLK = 512  # s-block width
NT = S // P  # 16 t tiles
NB = S // SBLK  # 4 s blocks / chunks
TPB = SBLK // P  # 4 t tiles per s block
SCALE = 1.0 / math.sqrt(D)

F32 = mybir.dt.float32
F32R = mybir.dt.float32r
BF16 = mybir.dt.float16  # fp16: same PE rate as bf16, 10-bit mantissa

# mm1 (QK^T) precision: False -> f32r (fp32 inputs), True -> bf16
MM1_BF16 = True


def build_nc(mm1_bf16=MM1_BF16):
    qk_np = np.float32 if not mm1_bf16 else np.float16
    qk_dt = F32 if not mm1_bf16 else BF16
    qk_sb_dt = F32R if not mm1_bf16 else BF16

    nc = bacc.Bacc("TRN2", target_bir_lowering=False, debug=False, num_devices=NCORES)
    qt_d = nc.dram_tensor("qt", [HC, D, S], qk_dt, kind="ExternalInput").ap()
    kt_d = nc.dram_tensor("kt", [HC, D, S], qk_dt, kind="ExternalInput").ap()
    v_d = nc.dram_tensor("v", [S, HC, D], BF16, kind="ExternalInput").ap()
    ot_d = nc.dram_tensor("ot", [HC, D, S], F32, kind="ExternalOutput").ap()

    with tile.TileContext(nc) as tc:
        with (
            tc.tile_pool(name="consts", bufs=1) as cpool,
            tc.tile_pool(name="big", bufs=1) as bigpool,
            tc.tile_pool(name="exp", bufs=8) as epool,
            tc.tile_pool(name="norm", bufs=3) as npool,
            tc.tile_pool(name="esum", bufs=3) as espool,
            tc.tile_pool(name="psum_s", bufs=2, space="PSUM") as ps_pool,
            tc.tile_pool(name="psum_o", bufs=3, space="PSUM") as po_pool,
            tc.tile_pool(name="psum_l", bufs=1, space="PSUM") as pl_pool,
        ):
            ones = cpool.tile([P, P], BF16, tag="ones")
            nc.vector.memset(ones, 1.0)
            warm_ps = pl_pool.tile([P, SBLK], F32, tag="pl", name="warm_ps")
            for w in range(40):
                nc.tensor.matmul(
                    warm_ps[:, :P],
                    ones[:],
                    ones[:],
                    start=True,
                    stop=True,
                    skip_group_check=True,
                )
            ones_f = cpool.tile([P, P], F32, tag="ones_f")
            nc.vector.memset(ones_f, 1.0)
            ones_r = cpool.tile([P, P], F32R, tag="ones_r")
            nc.vector.tensor_copy(out=ones_r[:], in_=ones_f[:])
            tri = cpool.tile([P, P], BF16, tag="tri")
            make_upper_triangular(nc, tri, val=1.0, diag=True)

            # chunked SBUF inputs: per-head K^T/Q^T [d, 512] chunks (qk_sb_dt)
            # and V natural [t-part, j, h, d] bf16 chunks, loaded in the order
            # compute consumes them.
            kt_c = {}
            qt_c = {}
            vb_c = {}
            vre = v_d.rearrange("(i p) h d -> p i h d", p=P)
            for c in range(NB):
                for h in range(HC):
                    kt_c[h, c] = bigpool.tile(
                        [P, SBLK], qk_sb_dt, tag=f"ktc{h}_{c}", name=f"ktc{h}_{c}"
                    )
                    qt_c[h, c] = bigpool.tile(
                        [P, SBLK], qk_sb_dt, tag=f"qtc{h}_{c}", name=f"qtc{h}_{c}"
                    )
                vb_c[c] = bigpool.tile(
                    [P, TPB, HC, D], BF16, tag=f"vbc{c}", name=f"vbc{c}"
                )
            # issue DMAs in the order blocks consume them, alternating issue
            # engines so descriptor writes don't serialize on one sequencer
            dma_jobs = []
            seen = set()

            def _need(key, dst, srcap):
                if key not in seen:
                    seen.add(key)
                    dma_jobs.append((dst, srcap))

            for b in (0, 1, 2, 3):
                for h in range(HC):
                    cs = slice(b * SBLK, (b + 1) * SBLK)
                    _need(("q", h, b), qt_c[h, b], qt_d[h, :, cs].bitcast(qk_sb_dt))
                    for c in range(b + 1):
                        ks = slice(c * SBLK, (c + 1) * SBLK)
                        _need(("k", h, c), kt_c[h, c], kt_d[h, :, ks].bitcast(qk_sb_dt))
                        _need(("v", c), vb_c[c], vre[:, c * TPB : (c + 1) * TPB])
            for n_dma, (dst, srcap) in enumerate(dma_jobs):
                eng = nc.sync
                if n_dma < 5 and len(dst.shape) == 2:
                    # split the first, compute-gating transfers across two
                    # queues each so the pipeline fills sooner
                    half = dst.shape[-1] // 2
                    eng.dma_start(dst[:, :half], srcap[:, :half])
                    eng.dma_start(dst[:, half:], srcap[:, half:])
                else:
                    eng.dma_start(dst[:], srcap)

            def kt_tile(h, i):
                return kt_c[h, i // TPB][:, (i % TPB) * P : (i % TPB + 1) * P]

            def v_tile(h, i):
                return vb_c[i // TPB][:, i % TPB, h, :]

            # smallest block first: it only needs the first input chunks, so
            # the pipeline fills at the earliest possible moment
            for b in (0, 1, 2, 3):
                n_full = TPB * b  # fully-unmasked t tiles (even count)
                # groups of two t tiles sharing one 2-bank psum + one exp:
                # (i0, i1, s_lo0, s_lo1, is_diag)
                groups = [(ip, ip + 1, 0, 0, False) for ip in range(0, n_full, 2)]
                groups += [
                    (n_full, n_full + 1, 0, P, True),
                    (n_full + 2, n_full + 3, 2 * P, 3 * P, True),
                ]
                n_groups = len(groups)
                last_i = n_full + TPB - 1

                psum_o = {}
                psum_l = {}
                expsum = {}
                expt_of = {}
                for h in range(HC):
                    psum_o[h] = po_pool.tile(
                        [P, SBLK], F32, tag="po", name=f"po{h}_{b}"
                    )
                    psum_l[h] = pl_pool.tile(
                        [P, SBLK], F32, tag="pl", name=f"pl{h}_{b}"
                    )
                    if n_full:
                        expsum[h] = bigpool.tile(
                            [P, SBLK], F32R, tag=f"esum{h}_{b}", name=f"es{h}_{b}"
                        )

                def emit_mm1(h, g):
                    i0, i1, s0, s1, is_diag = groups[g]
                    psum_s = ps_pool.tile(
                        [P, 2, SBLK], F32, tag="ps", name=f"ps{h}_{b}_{g}"
                    )
                    expt = epool.tile(
                        [P, 2, SBLK], BF16, tag="expt", name=f"ex{h}_{b}_{g}"
                    )
                    for j, (i, s_lo) in enumerate(((i0, s0), (i1, s1))):
                        nc.tensor.matmul(
                            psum_s[:, j, s_lo:],
                            kt_tile(h, i),
                            qt_c[h, b][:, s_lo:],
                            start=True,
                            stop=True,
                        )
                    # one exp for both tiles; [s0:s1] of tile 1 is stale-finite
                    # psum, never read downstream
                    nc.scalar.activation(
                        expt[:, :, s0:],
                        psum_s[:, :, s0:],
                        mybir.ActivationFunctionType.Exp,
                        scale=SCALE,
                    )
                    if is_diag:
                        for j, s_lo in enumerate((s0, s1)):
                            nc.gpsimd.tensor_mul(
                                out=expt[:, j, s_lo : s_lo + P],
                                in0=expt[:, j, s_lo : s_lo + P],
                                in1=tri[:],
                            )
                    expt_of[h, g] = expt

                def emit_mm2(h, g):
                    i0, i1, s0, s1, is_diag = groups[g]
                    expt = expt_of.pop((h, g))
                    for j, (i, s_lo) in enumerate(((i0, s0), (i1, s1))):
                        nc.tensor.matmul(
                            psum_o[h][:, s_lo:],
                            v_tile(h, i),
                            expt[:, j, s_lo:],
                            start=(i == 0),
                            stop=(i == last_i),
                            skip_group_check=True,
                        )
                        if is_diag:
                            # diagonal denominator contributions on PE
                            nc.tensor.matmul(
                                psum_l[h][:, s_lo:],
                                ones[:],
                                expt[:, j, s_lo:],
                                start=(i == n_full),
                                stop=(i == last_i and n_full == 0),
                                skip_group_check=True,
                            )
                    if not is_diag:
                        # full-tile denominator contributions accumulate on DVE
                        # (same-dtype inputs per op: bf16+bf16 -> fp32 pair sum,
                        # then fp32+fp32 accumulate)
                        if i0 == 0:
                            nc.vector.tensor_add(
                                out=expsum[h][:],
                                in0=expt[:, 0, :],
                                in1=expt[:, 1, :],
                            )
                        else:
                            psum_pair = npool.tile(
                                [P, SBLK], BF16, tag="epair", name=f"ep{h}_{b}_{i0}"
                            )
                            nc.vector.tensor_add(
                                out=psum_pair[:],
                                in0=expt[:, 0, :],
                                in1=expt[:, 1, :],
                            )
                            nc.vector.tensor_add(
                                out=expsum[h][:],
                                in0=expsum[h][:],
                                in1=psum_pair[:],
                            )

                # interleave the two heads' streams: PE runs head A's mm2
                # while ACT computes head B's exp
                pending = None
                for g in range(n_groups):
                    for h in range(HC):
                        emit_mm1(h, g)
                    if pending is not None:
                        for h in range(HC):
                            emit_mm2(h, pending)
                    pending = g
                for h in range(HC):
                    emit_mm2(h, pending)

                for h in range(HC):
                    bs = slice(b * SBLK, (b + 1) * SBLK)
                    if n_full:
                        # contract the DVE partial sums over the partition dim
                        nc.tensor.matmul(
                            psum_l[h][:],
                            ones_r[:],
                            expsum[h][:],
                            start=False,
                            stop=True,
                            skip_group_check=True,
                        )
                    recip = npool.tile([P, SBLK], F32, tag="recip", name=f"rc{h}_{b}")
                    nc.vector.reciprocal_approx_fast(out=recip[:], in_=psum_l[h][:])
                    otn = npool.tile([P, SBLK], F32, tag="otn", name=f"ot{h}_{b}")
                    nc.vector.tensor_mul(out=otn[:], in0=psum_o[h][:], in1=recip[:])
                    # split across two queues so the final transfer (which
                    # gates the exit drain) completes sooner
                    hw = SBLK // 2
                    lo = b * SBLK
                    nc.sync.dma_start(ot_d[h, :, lo : lo + hw], otn[:, :hw])
                    nc.sync.dma_start(ot_d[h, :, lo + hw : lo + SBLK], otn[:, hw:])
    nc.compile()
    return nc


_NC_CACHE = None


def _get_nc():
    global _NC_CACHE
    if _NC_CACHE is None:
        _NC_CACHE = build_nc()
    return _NC_CACHE


def make_in_maps(query, key, value):
    qk_np = np.float32 if not MM1_BF16 else np.float16
    query = np.asarray(query)
    key = np.asarray(key)
    value = np.asarray(value)
    in_maps = []
    for c in range(NCORES):
        hs = slice(c * HC, (c + 1) * HC)
        in_maps.append(
            {
                "qt": np.ascontiguousarray(
                    query[0, :, hs, :].transpose(1, 2, 0)
                ).astype(qk_np),
                "kt": np.ascontiguousarray(
                    key[0, :, hs, :].transpose(1, 2, 0)
                ).astype(qk_np),
                "v": np.ascontiguousarray(value[0, :, hs, :]).astype(
                    np.float16
                ),
            }
        )
    return in_maps


def kernel(query, key, value):
    from concourse.bass_utils import run_bass_kernel_spmd

    nc = _get_nc()
    in_maps = make_in_maps(query, key, value)
    res = run_bass_kernel_spmd(nc, in_maps, core_ids=list(range(NCORES)))
    out = np.empty((1, S, H, D), dtype=np.float32)
    for c in range(NCORES):
        # ot is [HC, D, S] -> [S, HC, D]
        out[0, :, c * HC : (c + 1) * HC, :] = res.results[c]["ot"].transpose(2, 0, 1)
    return out
